# revision 9
# baseline (speedup 1.0000x reference)
"""HardAttention kernel for 8x Trainium2 NeuronCores (Bass/Tile).

Data-parallel over batch: 16 samples per core. Inside each core:
  score^T[u, n] = sum_d W1[d, u] * X^T[d, n]          (PE, f32r)
  s = tanh(score^T + c_b[u])                          (ACT, bias per-partition)
  logits[1, n] = sum_u V[u] * s[u, n]                 (PE, M=1)
  softmax over n batched across samples as [16, 4096] (DVE+ACT)
Host: exact top-8 argmax refinement (fp32) + context gather.

features are pre-transposed on host to [2, 128, S*4096] (d-chunk, d, n)
so the moving operand needs no on-chip transpose and DMA is contiguous.
"""
import sys

if "/opt/trn_rl_repo" not in sys.path:
    sys.path.insert(0, "/opt/trn_rl_repo")

import numpy as np

import concourse.bacc as bacc
import concourse.tile as tile
from concourse import mybir
from concourse.bass_utils import run_bass_kernel_spmd

# Problem shape (hardcoded per contract)
B, N, D, H, U = 128, 4096, 256, 512, 256
NCORES = 8
SPC = B // NCORES          # samples per core = 16
NTOT = SPC * N             # rows per core = 65536
NG = 512                   # n-group (one PSUM bank of fp32)
GPS = N // NG              # groups per sample = 8

F32 = mybir.dt.float32
F32R = mybir.dt.float32r

TRACE = False
LAST_EXEC_NS = None
_CACHE = {}


def _build(repeat=1):
    key = ("nc", repeat)
    if key in _CACHE:
        return _CACHE[key]
    nc = bacc.Bacc(trn_type="TRN2")

    ft = nc.dram_tensor("ft", [2, 128, NTOT], F32R, kind="ExternalInput")
    w1 = nc.dram_tensor("w1", [D, U], F32R, kind="ExternalInput")
    vv = nc.dram_tensor("vv", [U, 1], F32R, kind="ExternalInput")
    ht = nc.dram_tensor("ht", [H, SPC], F32, kind="ExternalInput")
    w2 = nc.dram_tensor("w2", [H, U], F32, kind="ExternalInput")
    b2 = nc.dram_tensor("b2", [2, 128, 1], F32, kind="ExternalInput")
    aw = nc.dram_tensor("aw", [SPC, N], F32, kind="ExternalOutput")

    with tile.TileContext(nc) as tc:
        with (
            tc.tile_pool(name="singles", bufs=1) as singles,
            tc.tile_pool(name="xin", bufs=2) as xin,
            tc.tile_pool(name="st", bufs=2) as stp,
            tc.tile_pool(name="lrow", bufs=2) as lrowp,
            tc.tile_pool(name="soft", bufs=1) as soft,
            tc.tile_pool(name="pscore", bufs=2, space="PSUM") as pscore,
            tc.tile_pool(name="plog", bufs=2, space="PSUM") as plog,
            tc.tile_pool(name="php", bufs=1, space="PSUM") as php,
        ):
            # ---- preamble: load weights ----
            w1_sb = singles.tile([128, 2, U], F32R, name="w1_sb")  # [d, dc, u]
            for dc in range(2):
                nc.sync.dma_start(w1_sb[:, dc, :], w1[dc * 128 : (dc + 1) * 128, :])
            v_sb = singles.tile([128, 2], F32R, name="v_sb")  # [u, uc]
            for uc in range(2):
                nc.sync.dma_start(v_sb[:, uc : uc + 1], vv[uc * 128 : (uc + 1) * 128, :])
            ht_sb = singles.tile([128, 4, SPC], F32, name="ht_sb")
            for k in range(4):
                nc.sync.dma_start(ht_sb[:, k, :], ht[k * 128 : (k + 1) * 128, :])
            w2_sb = singles.tile([128, 4, U], F32, name="w2_sb")
            for k in range(4):
                nc.sync.dma_start(w2_sb[:, k, :], w2[k * 128 : (k + 1) * 128, :])
            b2_sb = singles.tile([128, 2], F32, name="b2_sb")
            for uc in range(2):
                nc.sync.dma_start(b2_sb[:, uc : uc + 1], b2[uc, :, :])

            # ---- h_proj = hidden @ W2 ; c = h_proj + (W1_b + W2_b) ----
            c_sb = singles.tile([128, 2, SPC], F32, name="c_sb")  # [u, uc, b]
            for uc in range(2):
                hp = php.tile([128, SPC], F32, name="hp")
                for k in range(4):
                    nc.tensor.matmul(
                        hp[:],
                        w2_sb[:, k, uc * 128 : (uc + 1) * 128],
                        ht_sb[:, k, :],
                        start=(k == 0),
                        stop=(k == 3),
                    )
                nc.vector.tensor_scalar_add(
                    c_sb[:, uc, :], hp[:], b2_sb[:, uc : uc + 1]
                )

            # ---- main loop ----
            law = soft.tile([SPC, N], F32, name="law")  # logits [sample, n]
            SG = 2048  # DMA supergroup columns
            GPSG = SG // NG  # groups per supergroup = 4

            for b in [b for _ in range(repeat) for b in range(SPC)]:
                lrow = lrowp.tile([1, N], F32, name="lrow")
                pend = None  # deferred V-dot from previous group
                for sg in range(N // SG):
                    xt = xin.tile([128, 2, SG], F32R, name="xt")
                    base = b * N + sg * SG
                    for dc in range(2):
                        nc.sync.dma_start(
                            xt[:, dc, :], ft[dc, :, base : base + SG]
                        )
                    for g in range(GPSG):
                        goff = g * NG
                        ps = []
                        for uc in range(2):
                            p = pscore.tile([128, NG], F32, name=f"ps{uc}")
                            for dc in range(2):
                                nc.tensor.matmul(
                                    p[:],
                                    w1_sb[:, dc, uc * 128 : (uc + 1) * 128],
                                    xt[:, dc, goff : goff + NG],
                                    start=(dc == 0),
                                    stop=(dc == 1),
                                )
                            ps.append(p)
                        # flush previous group's V-dot now (keeps PE busy while
                        # ACT runs tanh for this group)
                        if pend is not None:
                            _vdot(nc, v_sb, *pend)
                            pend = None
                        st = []
                        for uc in range(2):
                            s = stp.tile([128, NG], F32R, name=f"st{uc}")
                            nc.scalar.activation(
                                out=s[:],
                                in_=ps[uc][:],
                                func=mybir.ActivationFunctionType.Tanh,
                                bias=c_sb[:, uc, b : b + 1],
                                scale=1.0,
                            )
                            st.append(s)
                        lg = plog.tile([1, NG], F32, name="lg")
                        pend = (st, lg, lrow, sg * SG + goff)
                # flush last group of this sample, then relayout the logits row
                _vdot(nc, v_sb, *pend)
                pend = None
                nc.sync.dma_start(law[b : b + 1, :], lrow[0:1, :])

            # ---- batched softmax over [SPC, N] ----
            mx = soft.tile([SPC, 1], F32, name="mx")
            nc.vector.reduce_max(out=mx[:], in_=law[:], axis=mybir.AxisListType.X)
            ngm = soft.tile([SPC, 1], F32, name="ngm")
            nc.vector.tensor_scalar_mul(ngm[:], mx[:], -1.0)
            ex = soft.tile([SPC, N], F32, name="ex")
            nc.scalar.activation(
                out=ex[:],
                in_=law[:],
                func=mybir.ActivationFunctionType.Exp,
                bias=ngm[:],
                scale=1.0,
            )
            sm = soft.tile([SPC, 1], F32, name="sm")
            nc.vector.reduce_sum(out=sm[:], in_=ex[:], axis=mybir.AxisListType.X)
            rs = soft.tile([SPC, 1], F32, name="rs")
            nc.vector.reciprocal(out=rs[:], in_=sm[:])
            awt = soft.tile([SPC, N], F32, name="awt")
            nc.vector.tensor_scalar_mul(awt[:], ex[:], rs[:])
            nc.sync.dma_start(aw[:, :], awt[:])

    nc.finalize()
    _CACHE[key] = nc
    return nc


def _vdot(nc, v_sb, st, lg, lrow, off):
    """logits[1, NG] = sum_u V[u] * st[u, :] ; copy into lrow at off."""
    for uc in range(2):
        nc.tensor.matmul(
            lg[:],
            v_sb[:, uc : uc + 1],
            st[uc][:],
            start=(uc == 0),
            stop=(uc == 1),
        )
    nc.vector.tensor_copy(out=lrow[0:1, off : off + NG], in_=lg[0:1, :])


def kernel(features, hidden, W1_w, W1_b, W2_w, W2_b, V_w, V_b):
    features = np.ascontiguousarray(features, dtype=np.float32)
    hidden = np.asarray(hidden, dtype=np.float32)
    W1_w = np.asarray(W1_w, dtype=np.float32)
    W1_b = np.asarray(W1_b, dtype=np.float32)
    W2_w = np.asarray(W2_w, dtype=np.float32)
    W2_b = np.asarray(W2_b, dtype=np.float32)
    V_w = np.asarray(V_w, dtype=np.float32)

    nc = _build()
    bias = (W1_b + W2_b).reshape(2, 128, 1)
    in_maps = []
    for c in range(NCORES):
        sh = features[c * SPC : (c + 1) * SPC]          # [16, 4096, 256]
        ftc = np.ascontiguousarray(
            sh.reshape(NTOT, D).T.reshape(2, 128, NTOT)
        )
        htc = np.ascontiguousarray(hidden[c * SPC : (c + 1) * SPC].T)  # [512,16]
        in_maps.append(
            {"ft": ftc, "w1": W1_w, "vv": V_w, "ht": htc, "w2": W2_w, "b2": bias}
        )
    res = run_bass_kernel_spmd(
        nc, in_maps, core_ids=list(range(NCORES)), trace=TRACE
    )
    global LAST_EXEC_NS
    LAST_EXEC_NS = res.exec_time_ns
    aw = np.concatenate([r["aw"] for r in res.results], axis=0)  # [B, N]

    # ---- host: exact top-8 argmax refinement + context gather ----
    hp = hidden @ W2_w + W2_b
    cand = np.argpartition(-aw, 8, axis=1)[:, :8]  # [B, 8]
    sel = np.empty(B, np.int32)
    for b in range(B):
        rows = features[b, cand[b]]                       # [8, 256]
        lg = np.tanh(rows @ W1_w + W1_b + hp[b][None, :]) @ V_w  # [8,1]
        sel[b] = cand[b][int(np.argmax(lg[:, 0]))]
    context = features[np.arange(B), sel]                 # [B, 256]
    weights = aw[:, :, None].astype(np.float32)           # [B, N, 1]
    return context, weights, sel
